# revision 13
# baseline (speedup 1.0000x reference)
"""Trainium2 Bass kernel for ConvTranspose4d (T: 3-tap valid conv; D/H/W:
stride-2 k=3 p=1 transposed conv). Self-contained: hardcoded shapes.

x: [1, 8, 8, 24, 48, 48] f32, weight: [8, 8, 3, 3, 3, 3] f32
out: [1, 8, 6, 47, 95, 95] f32

Strategy (8 NeuronCores, data-parallel over D):
  - Core j computes output od = 6j..6j+5 (core 7 drops od 47); needs input
    slices id0..id0+3 (id0 = min(3j, 20)).
  - Temporal 3-tap conv and D-axis stride-2 transposed conv fold into the
    matmul stationary operand as a banded weight matrix:
      lhsT[K=128=(slot4, cin8, id4), M=(fbit2 x 48=(cout8*6+od))]
    where slot = kt + fbit holds temporal plane 2i+slot of frame-pair i.
  - All I/O is bf16 (host converts): input planes pre-laid-out per
    frame-pair as [3, 128, 49, 49] (zero-padded), triple-buffered in SBUF
    so the PE never stalls on WAR hazards; bands are [128, 9, 128].
  - H/W parities are 4 output classes (ph, pw); each accumulates 1/2/2/4
    shifted-view taps in PSUM (kh = ph - 2*dh + 1).
  - Staging is PARITY-PLANAR: each class region is contiguous per
    partition, so PSUM->SBUF copies (DVE for ph=0, ACT for ph=1) are
    contiguous; the host interleaves the stride-2 H/W grids during the
    gather. Output DMA (bf16) is split across both HWDGE queues.
"""
import numpy as np

COMPUTE = "bfloat16"

TAPS = {
    (0, 0): [(0, 0)],
    (0, 1): [(0, 0), (0, 1)],
    (1, 0): [(0, 0), (1, 0)],
    (1, 1): [(0, 0), (0, 1), (1, 0), (1, 1)],
}
TAP_LIST = [(ph, pw, dh, dw) for (ph, pw), tl in TAPS.items() for (dh, dw) in tl]
CHUNK_START = [0, 10, 20, 30, 40]
CHUNK_N = [10, 10, 10, 10, 8]
PAIRS = [(0, 1), (2, 3), (4,)]
# class -> (region offset in stg, rows, cols)
REGION = {
    (0, 0): (0, 48, 48),
    (0, 1): (2304, 48, 47),
    (1, 0): (4560, 47, 48),
    (1, 1): (6816, 47, 47),
}

_CACHE = {}


def _bf16():
    import ml_dtypes
    return ml_dtypes.bfloat16


def _build_bands(W, j):
    """W: [cin8, cout8, kt3, kd3, kh3, kw3] -> [128, 9, 128] f32.
    K row = slot*32 + cin*4 + id (slot = kt + fbit);
    M col = fbit*48 + cout*6 + od (od 0..5; cols 96..127 zero)."""
    id0 = min(3 * j, 20)
    B = np.zeros((128, 9, 128), np.float32)
    ci = np.arange(8)
    co = np.arange(8)
    for t, (ph, pw, dh, dw) in enumerate(TAP_LIST):
        kh = ph - 2 * dh + 1
        kw = pw - 2 * dw + 1
        for fbit in range(2):
            for kt in range(3):
                slot = kt + fbit
                for idl in range(4):
                    for od in range(6):
                        od_g = 6 * j + od
                        if od_g > 46:
                            continue
                        kd = od_g - 2 * (id0 + idl) + 1
                        if not (0 <= kd <= 2):
                            continue
                        krow = slot * 32 + ci * 4 + idl
                        mcol = fbit * 48 + co * 6 + od
                        B[krow[:, None], t, mcol[None, :]] = W[:, :, kt, kd, kh, kw]
    return B


def _free_view(base, off, dims):
    """Hand-built AP: keep base's partition dim, replace free dims with
    [(step, count), ...] (element units) at extra offset `off`."""
    a = base.copy()
    v = a.ap
    part = v.to_list()[0]
    v.clear()
    v.append(part)
    for sc in dims:
        v.append(list(sc))
    a.ap = v
    a.offset = a.offset + off
    return a


def _build_program():
    import concourse.bacc as bacc
    import concourse.tile as tile
    from concourse import mybir

    f32 = mybir.dt.float32
    bf16 = mybir.dt.bfloat16

    nc = bacc.Bacc("TRN2", target_bir_lowering=False, debug=False)
    xs_ap = nc.dram_tensor("xs", [3, 128, 2401], bf16, kind="ExternalInput").ap()
    bd_ap = nc.dram_tensor("bands", [128, 9, 128], bf16, kind="ExternalInput").ap()
    out_ap = nc.dram_tensor("out", [8, 6, 6, 9025], bf16, kind="ExternalOutput").ap()

    with tile.TileContext(nc, trace_sim=False) as tc:
        with (
            tc.tile_pool(name="xp", bufs=3) as xp,
            tc.tile_pool(name="bp", bufs=1) as bp,
            tc.tile_pool(name="sp", bufs=3) as sp,
            tc.tile_pool(name="ps", bufs=8, space="PSUM") as ps,
        ):
            bt = bp.tile([128, 9, 128], bf16)
            nc.scalar.dma_start(out=bt[:], in_=bd_ap)

            # PE warm-up: dense dummy matmuls while the input DMAs land, so
            # the HAM clock-gate opens (K=8/8) before the real stream starts.
            dz = bp.tile([128, 64], bf16)
            nc.vector.memset(dz[:], 0.0)
            wps = ps.tile([128, 512], f32, name="warm", tag="ps")
            for _ in range(56):
                nc.tensor.matmul(wps[0:64, 0:64], dz[:], dz[:], start=True,
                                 stop=True)

            xts = []
            for i in range(3):
                xt = xp.tile([128, 49, 49], bf16, name=f"xt{i}", tag="xt")
                nc.sync.dma_start(
                    out=xt[:].rearrange("p h w -> p (h w)"), in_=xs_ap[i]
                )
                xts.append(xt)

            for i in range(3):
                f0 = 2 * i
                xt = xts[i]
                stg = sp.tile([128, 9025], bf16, name=f"stg{i}", tag="stg")
                for (ph, pw) in TAPS:
                    taps = TAPS[(ph, pw)]
                    roff, _, nmw = REGION[(ph, pw)]
                    for c in range(5):
                        pt = ps.tile([128, 512], f32, name="ps", tag="ps")
                        mh0, nmh = CHUNK_START[c], CHUNK_N[c]
                        for ti, (dh, dw) in enumerate(taps):
                            t_idx = TAP_LIST.index((ph, pw, dh, dw))
                            lhsT = bt[:, t_idx, :]
                            rhs = xt[:, mh0 + dh:mh0 + dh + nmh, dw:dw + 48]
                            nc.tensor.matmul(
                                pt[:, 0:nmh * 48], lhsT, rhs,
                                start=(ti == 0), stop=(ti == len(taps) - 1),
                            )
                        # contiguous copy PSUM -> class-planar staging
                        nmh_c = nmh if c < 4 else CHUNK_N[4] - ph
                        src = _free_view(pt[0:96], 0, [(48, nmh_c), (1, nmw)])
                        doff = roff + CHUNK_START[c] * nmw
                        dst = _free_view(stg[0:96], doff,
                                         [(nmw, nmh_c), (1, nmw)])
                        if ph == 0:
                            nc.vector.tensor_copy(dst, src)
                        else:
                            nc.scalar.copy(dst, src)
                # output DMA: per (fbit, ph-half) = 48 descriptors each,
                # spread over sync HWDGE + gpsimd software queues
                for fbit in range(2):
                    for phalf in range(2):
                        r0 = 0 if phalf == 0 else 4560
                        r1 = 4560 if phalf == 0 else 9025
                        eng = nc.sync if (fbit ^ phalf) == 0 else nc.gpsimd
                        eng.dma_start(
                            out=out_ap[:, f0 + fbit, :, r0:r1],
                            in_=_free_view(
                                stg[fbit * 48:fbit * 48 + 48], r0,
                                [(1, r1 - r0)]),
                        )

    nc.compile()
    return nc


def _get_program():
    if "nc" not in _CACHE:
        _CACHE["nc"] = _build_program()
    return _CACHE["nc"]


def run(x, weight, trace=False):
    from concourse.bass_utils import run_bass_kernel_spmd

    bf16 = _bf16()
    x = np.asarray(x, dtype=np.float32)
    weight = np.asarray(weight, dtype=np.float32)
    in_maps = []
    for j in range(8):
        id0 = min(3 * j, 20)
        xs = np.zeros((3, 4, 8, 4, 49, 49), np.float32)
        for i in range(3):
            for slot in range(4):
                # [c, id, 48, 48]
                xs[i, slot, :, :, :48, :48] = x[0, :, 2 * i + slot, id0:id0 + 4]
        # partition = slot*32 + c*4 + idl  ->  order [i, slot, c, idl, h, w]
        xs = xs.reshape(3, 128, 2401)
        in_maps.append({
            "xs": xs.astype(bf16),
            "bands": _build_bands(weight, j).astype(bf16),
        })
    nc = _get_program()
    res = run_bass_kernel_spmd(nc, in_maps, core_ids=list(range(8)), trace=trace)
    full = np.zeros((1, 8, 6, 47, 95, 95), np.float32)
    for j in range(8):
        nod = min(6, 47 - 6 * j)
        oj = np.asarray(res.results[j]["out"]).astype(np.float32)
        oj = oj[:, :, :nod]  # [8, 6, nod, 9025]
        dst = full[0, :, :, 6 * j:6 * j + nod]
        for (ph, pw), (roff, nr, ncol) in REGION.items():
            dst[..., ph::2, pw::2] = oj[..., roff:roff + nr * ncol].reshape(
                8, 6, nod, nr, ncol)
    return full, res


def kernel(x, weight):
    return run(x, weight)[0]
